# revision 4
# baseline (speedup 1.0000x reference)
"""AdditiveClassifier (attention-gated MLP + per-bag segment sum) on 8 TRN2 cores.

Sharding: patch dim N=131072 split as 16384 rows/core (whole bags stay on one
core). MLP weights replicated. Per-core:
  xT tiles are produced on-chip (fp32->bf16 cast during DMA, then X-bar
  SBUF->SBUF transposes), L1 = W1^T-stationary bf16 matmuls accumulating in
  PSUM as h^T [H,rows], gate by attention (broadcast along partitions) + bias
  + ReLU on DVE/ACT, L2 = small bf16 matmul -> patch_logits^T [2,rows],
  bias on DVE, block-transpose trick (32x32 StreamTranspose) for the natural
  [rows,2] store, per-chunk free-dim reductions for the bag sums.
Outputs gathered on host by concatenation (no collectives needed).
"""
import os
import numpy as np
import ml_dtypes

import concourse.bacc as bacc
import concourse.mybir as mybir
import concourse.tile as tile
from concourse.bass_utils import run_bass_kernel_spmd

N_CORES = 8
N, D, H, C = 131072, 1024, 512, 2
ROWS = N // N_CORES          # 16384 rows per core
R = 512                      # rows per chunk (PSUM free-dim limit for fp32)
CHUNKS = ROWS // R           # 32
KO1 = D // 128               # 8  k-tiles for L1
MO1 = H // 128               # 4  m-tiles for L1 (= k-tiles for L2)
RT = R // 128                # 4  row-tiles per chunk

CAST_MODE = os.environ.get("AC_CAST_MODE", "swdge")   # swdge | dve
TRANSPOSE_SPLIT = os.environ.get("AC_TSPLIT", "1") == "1"  # transposes on 2 rings

_built = None


def build():
    global _built
    if _built is not None:
        return _built
    nc = bacc.Bacc("TRN2", target_bir_lowering=False, debug=False,
                   num_devices=N_CORES)
    f32, bf16 = mybir.dt.float32, mybir.dt.bfloat16

    x = nc.declare_dram_parameter("x", [ROWS, D], f32, isOutput=False)
    att = nc.declare_dram_parameter("att", [ROWS], f32, isOutput=False)
    w1 = nc.declare_dram_parameter("w1", [D, H], bf16, isOutput=False)
    b1 = nc.declare_dram_parameter("b1", [H], f32, isOutput=False)
    w2 = nc.declare_dram_parameter("w2", [H, C], bf16, isOutput=False)
    b2 = nc.declare_dram_parameter("b2", [C], f32, isOutput=False)
    pl = nc.declare_dram_parameter("pl", [ROWS, C], f32, isOutput=True)
    lg = nc.declare_dram_parameter("lg", [ROWS // 2048, C], f32, isOutput=True)

    x_chunks = x[:, :].rearrange("(c rt p) d -> c p rt d", p=128, rt=RT)

    with tile.TileContext(nc) as tc:
        with (
            tc.tile_pool(name="const", bufs=1) as constp,
            tc.tile_pool(name="nat", bufs=3) as natp,
            tc.tile_pool(name="natf", bufs=2) as natfp,
            tc.tile_pool(name="xt", bufs=3) as xtp,
            tc.tile_pool(name="ab", bufs=2) as abp,
            tc.tile_pool(name="tg", bufs=2) as tgp,
            tc.tile_pool(name="ht", bufs=8) as htp,
            tc.tile_pool(name="plt", bufs=2) as pltp,
            tc.tile_pool(name="ps1", bufs=4, space="PSUM") as ps1p,
            tc.tile_pool(name="ps2", bufs=2, space="PSUM") as ps2p,
        ):
            # ---- preload constants ----
            w1sb = constp.tile([128, KO1, H], bf16)
            for ko in range(KO1):
                nc.scalar.dma_start(w1sb[:, ko, :], w1[ko * 128:(ko + 1) * 128, :])
            b1sb = constp.tile([128, MO1], f32)
            nc.scalar.dma_start(b1sb[:], b1[:].rearrange("(mo p) -> p mo", p=128))
            w2sb = constp.tile([128, MO1, C], bf16)
            nc.scalar.dma_start(w2sb[:], w2[:, :].rearrange("(ko ki) c -> ki ko c", ki=128))
            b2sb = constp.tile([C, 1], f32)
            nc.scalar.dma_start(b2sb[:], b2[:].rearrange("(c one) -> c one", one=1))
            psb = constp.tile([C, CHUNKS], f32)

            for c in range(CHUNKS):
                r0 = c * R
                # ---- natural-layout load with fp32->bf16 cast ----
                xnat = natp.tile([128, RT, D], bf16, tag="nat")
                if CAST_MODE == "swdge":
                    nc.gpsimd.dma_start(xnat[:], x_chunks[c])
                else:
                    xf32 = natfp.tile([128, RT, D], f32, tag="natf")
                    nc.sync.dma_start(xf32[:], x_chunks[c])
                    nc.vector.tensor_copy(xnat[:], xf32[:])
                # ---- on-chip transpose to xT [128d, KO1, R] ----
                xt = xtp.tile([128, KO1, R], bf16, tag="xt")
                for rt in range(RT):
                    for kt in range(KO1):
                        eng = nc.scalar if (TRANSPOSE_SPLIT and (kt % 2)) else nc.sync
                        eng.dma_start(
                            xt[:, kt, rt * 128:(rt + 1) * 128],
                            xnat[:, rt, kt * 128:(kt + 1) * 128],
                            transpose=True,
                        )
                # ---- attention broadcast [128, R] ----
                ab = abp.tile([128, R], f32, tag="ab")
                nc.scalar.dma_start(ab[:], att[r0:r0 + R].partition_broadcast(128))
                # ---- L1 + gate + ReLU -> hT bf16 tiles ----
                ht = []
                for mo in range(MO1):
                    ps = ps1p.tile([128, R], f32, tag="ps")
                    for ko in range(KO1):
                        nc.tensor.matmul(
                            ps[:],
                            w1sb[:, ko, mo * 128:(mo + 1) * 128],
                            xt[:, ko, :],
                            start=(ko == 0), stop=(ko == KO1 - 1),
                        )
                    tg = tgp.tile([128, R], f32, tag="tg")
                    nc.vector.tensor_tensor(tg[:], ps[:], ab[:], op=mybir.AluOpType.mult)
                    h = htp.tile([128, R], bf16, tag="ht")
                    nc.scalar.activation(h[:], tg[:], mybir.ActivationFunctionType.Relu,
                                         bias=b1sb[:, mo:mo + 1])
                    ht.append(h)
                # ---- L2 -> patch_logits^T [2, R] ----
                ps2 = ps2p.tile([C, R], f32, tag="ps2")
                for ko in range(MO1):
                    nc.tensor.matmul(ps2[:], w2sb[:, ko, :], ht[ko][:],
                                     start=(ko == 0), stop=(ko == MO1 - 1))
                plt = pltp.tile([32, R], f32, tag="plt")
                nc.vector.memset(plt[:], 0.0)
                nc.vector.tensor_scalar_add(plt[0:C, :], ps2[:], b2sb[:])
                plb = pltp.tile([32, R], f32, tag="plb")
                nc.vector.transpose(plb[:], plt[:])
                # natural [R, 2] store via the 32x32-block layout
                nc.scalar.dma_start(
                    pl[r0:r0 + R, :].rearrange("(j rr) c -> rr j c", rr=32),
                    plb[:, :].rearrange("p (j cc) -> p j cc", cc=32)[:, :, 0:C],
                )
                # per-chunk bag partial
                nc.vector.reduce_sum(psb[:, c:c + 1], plt[0:C, :],
                                     axis=mybir.AxisListType.X)

            # ---- bag sums: 4 chunks per bag ----
            lgt = constp.tile([C, ROWS // 2048], f32)
            nc.vector.reduce_sum(lgt[:], psb[:, :].rearrange("p (b f) -> p b f", f=4),
                                 axis=mybir.AxisListType.X)
            nc.scalar.dma_start(lg[:, :].rearrange("b c -> c b"), lgt[:])

    nc.compile()
    _built = nc
    return nc


def _host_prep(features, attention, W1, b1, W2, b2):
    features = np.ascontiguousarray(np.asarray(features, dtype=np.float32))
    att = np.ascontiguousarray(np.asarray(attention, dtype=np.float32).reshape(-1))
    w1b = np.asarray(W1, dtype=np.float32).astype(ml_dtypes.bfloat16)
    w2b = np.asarray(W2, dtype=np.float32).astype(ml_dtypes.bfloat16)
    b1f = np.asarray(b1, dtype=np.float32)
    b2f = np.asarray(b2, dtype=np.float32)
    in_maps = []
    for cix in range(N_CORES):
        s = slice(cix * ROWS, (cix + 1) * ROWS)
        in_maps.append({"x": features[s], "att": att[s], "w1": w1b, "b1": b1f,
                        "w2": w2b, "b2": b2f})
    return in_maps


def kernel(features, attention, bag_sizes, W1, b1, W2, b2):
    nc = build()
    in_maps = _host_prep(features, attention, W1, b1, W2, b2)
    results = run_bass_kernel_spmd(nc, in_maps, core_ids=list(range(N_CORES))).results
    patch_logits = np.concatenate([r["pl"] for r in results], axis=0)
    bag_sizes = np.asarray(bag_sizes).reshape(-1).astype(np.int64)
    if bag_sizes.size and (bag_sizes == 2048).all():
        logits = np.concatenate([r["lg"] for r in results], axis=0)
    else:
        # general (never hit for this problem's spec): host segment-sum
        offsets = np.zeros(bag_sizes.size, dtype=np.int64)
        np.cumsum(bag_sizes[:-1], out=offsets[1:])
        logits = np.add.reduceat(patch_logits.astype(np.float64), offsets, axis=0).astype(np.float32)
    return logits.astype(np.float32), patch_logits.astype(np.float32)


# revision 31
# speedup vs baseline: 297.5726x; 297.5726x over previous
"""AdditiveClassifier (attention-gated MLP + per-bag segment sum) on 8 TRN2 cores.

Sharding: patch dim N=131072 split as 16384 rows/core (whole bags stay on one
core). MLP weights replicated. Per-core:
  xT tiles are produced on-chip (fp32->bf16 cast during DMA, then X-bar
  SBUF->SBUF transposes), L1 = W1^T-stationary bf16 matmuls accumulating in
  PSUM as h^T [H,rows], gate by attention (broadcast along partitions) + bias
  + ReLU on DVE/ACT, L2 = small bf16 matmul -> patch_logits^T [2,rows],
  bias on DVE, block-transpose trick (32x32 StreamTranspose) for the natural
  [rows,2] store, per-chunk free-dim reductions for the bag sums.
Outputs gathered on host by concatenation (no collectives needed).
"""
import os
import numpy as np
import ml_dtypes

import concourse.bacc as bacc
import concourse.mybir as mybir
import concourse.tile as tile
from concourse.bass_utils import run_bass_kernel_spmd
from concourse.masks import make_identity

N_CORES = 8
N, D, H, C = 131072, 1024, 512, 2
ROWS = N // N_CORES          # 16384 rows per core
R = 512                      # rows per chunk (PSUM free-dim limit for fp32)
CHUNKS = ROWS // R           # 32
KO1 = D // 128               # 8  k-tiles for L1
MO1 = H // 128               # 4  m-tiles for L1 (= k-tiles for L2)
RT = R // 128                # 4  row-tiles per chunk

CAST_MODE = os.environ.get("AC_CAST_MODE", "swdge")   # swdge | dve
N_XBAR_KT = int(os.environ.get("AC_XBAR_KT", "0"))    # kt groups on the X-bar path

_built = {}


def build(loop_rep=0):
    """loop_rep>0: wrap the chunk loop in a constant-bound For_i repeating it
    loop_rep times (on-device timing); 0: plain production build."""
    if loop_rep in _built:
        return _built[loop_rep]
    nc = bacc.Bacc("TRN2", target_bir_lowering=False, debug=False,
                   num_devices=N_CORES)
    f32, bf16 = mybir.dt.float32, mybir.dt.bfloat16

    x = nc.declare_dram_parameter("x", [ROWS, D], f32, isOutput=False)
    att = nc.declare_dram_parameter("att", [ROWS], f32, isOutput=False)
    w1 = nc.declare_dram_parameter("w1", [D, H], bf16, isOutput=False)
    b1 = nc.declare_dram_parameter("b1", [H], f32, isOutput=False)
    w2 = nc.declare_dram_parameter("w2", [H, C], bf16, isOutput=False)
    b2 = nc.declare_dram_parameter("b2", [C], f32, isOutput=False)
    # outputs stored transposed ([C, rows]) for contiguous per-partition DMA;
    # host transposes back during the gather
    pl = nc.declare_dram_parameter("pl", [C, ROWS], f32, isOutput=True)
    lg = nc.declare_dram_parameter("lg", [C, ROWS // 2048], f32, isOutput=True)
    chain_in = nc.declare_dram_parameter("chain", [1, 1], f32, isOutput=False)
    chain_out = nc.declare_dram_parameter("chain_o", [1, 1], f32, isOutput=True)

    x_chunks = x[:, :].rearrange("(c rt p) d -> c p rt d", p=128, rt=RT)

    with tile.TileContext(nc) as tc:
        with (
            tc.tile_pool(name="const", bufs=1) as constp,
            tc.tile_pool(name="nat", bufs=3) as natp,
            tc.tile_pool(name="natf", bufs=2) as natfp,
            tc.tile_pool(name="xt", bufs=3) as xtp,
            tc.tile_pool(name="ab", bufs=2) as abp,
            tc.tile_pool(name="tg", bufs=2) as tgp,
            tc.tile_pool(name="ht", bufs=8) as htp,
            tc.tile_pool(name="plt", bufs=2) as pltp,
            tc.tile_pool(name="ps1", bufs=4, space="PSUM") as ps1p,
            tc.tile_pool(name="ps2", bufs=1, space="PSUM") as ps2p,
            tc.tile_pool(name="tps", bufs=3, space="PSUM") as tpsp,
        ):
            # ---- preload constants ----
            w1sb = constp.tile([128, KO1, H], bf16)
            for ko in range(KO1):
                nc.scalar.dma_start(w1sb[:, ko, :], w1[ko * 128:(ko + 1) * 128, :])
            b1sb = constp.tile([128, MO1], f32)
            nc.scalar.dma_start(b1sb[:], b1[:].rearrange("(mo p) -> p mo", p=128))
            w2sb = constp.tile([128, MO1, C], bf16)
            nc.scalar.dma_start(w2sb[:], w2[:, :].rearrange("(ko ki) c -> ki ko c", ki=128))
            b2sb = constp.tile([C, 1], f32)
            nc.scalar.dma_start(b2sb[:], b2[:].rearrange("(c one) -> c one", one=1))
            ident = constp.tile([128, 128], bf16)
            make_identity(nc, ident[:])
            psb = constp.tile([C, CHUNKS], f32)
            cht = constp.tile([1, 1], f32)
            nc.scalar.dma_start(cht[:], chain_in[:, :])
            nc.scalar.dma_start(chain_out[:, :], cht[:])

            if loop_rep:
                loop_cm = tc.For_i(0, loop_rep, 1)
                loop_cm.__enter__()

            for c in range(CHUNKS):
                r0 = c * R
                # ---- natural-layout load with fp32->bf16 cast ----
                xnat = natp.tile([128, RT, D], bf16, tag="nat")
                if CAST_MODE == "swdge":
                    nc.gpsimd.dma_start(xnat[:], x_chunks[c])
                else:
                    xf32 = natfp.tile([128, RT, D], f32, tag="natf")
                    nc.sync.dma_start(xf32[:], x_chunks[c])
                    nc.vector.tensor_copy(xnat[:], xf32[:])
                # ---- on-chip transpose to xT [128d, KO1, R] via PE ----
                # 4 row-tile transposes of one kt land in one PSUM tile, then a
                # single wide copy moves them to SBUF (alternating DVE/ACT).
                xt = xtp.tile([128, KO1, R], bf16, tag="xt")
                for kt in range(KO1):
                    if kt >= KO1 - N_XBAR_KT:
                        # X-bar path: straight SBUF->SBUF transposed DMA
                        for rt in range(RT):
                            nc.scalar.dma_start(
                                xt[:, kt, rt * 128:(rt + 1) * 128],
                                xnat[:, rt, kt * 128:(kt + 1) * 128],
                                transpose=True)
                        continue
                    tps = tpsp.tile([128, RT, 128], bf16, tag="tps")
                    for rt in range(RT):
                        nc.tensor.matmul(
                            tps[:, rt, :], xnat[:, rt, kt * 128:(kt + 1) * 128],
                            ident[:], is_transpose=True, skip_group_check=True)
                    dst = xt[:, kt, :]
                    if kt % 2:
                        nc.scalar.activation(
                            dst, tps[:], mybir.ActivationFunctionType.Copy)
                    else:
                        nc.vector.tensor_copy(dst, tps[:])
                # ---- attention broadcast [128, R] ----
                ab = abp.tile([128, R], f32, tag="ab")
                nc.sync.dma_start(ab[:], att[r0:r0 + R].partition_broadcast(128))
                # ---- L1 + gate + ReLU -> hT bf16 tiles ----
                ht = []
                for mo in range(MO1):
                    ps = ps1p.tile([128, R], f32, tag="ps")
                    for ko in range(KO1):
                        nc.tensor.matmul(
                            ps[:],
                            w1sb[:, ko, mo * 128:(mo + 1) * 128],
                            xt[:, ko, :],
                            start=(ko == 0), stop=(ko == KO1 - 1),
                        )
                    tg = tgp.tile([128, R], f32, tag="tg")
                    nc.vector.tensor_tensor(tg[:], ps[:], ab[:], op=mybir.AluOpType.mult)
                    h = htp.tile([128, R], bf16, tag="ht")
                    nc.scalar.activation(h[:], tg[:], mybir.ActivationFunctionType.Relu,
                                         bias=b1sb[:, mo:mo + 1])
                    ht.append(h)
                # ---- L2 -> patch_logits^T [2, R] ----
                ps2 = ps2p.tile([C, R], f32, tag="ps2")
                for ko in range(MO1):
                    nc.tensor.matmul(ps2[:], w2sb[:, ko, :], ht[ko][:],
                                     start=(ko == 0), stop=(ko == MO1 - 1))
                plt = pltp.tile([C, R], f32, tag="plt")
                nc.vector.tensor_scalar_add(plt[:], ps2[:], b2sb[:])
                nc.sync.dma_start(pl[:, r0:r0 + R], plt[:])
                # per-chunk bag partial
                nc.vector.reduce_sum(psb[:, c:c + 1], plt[:],
                                     axis=mybir.AxisListType.X)

            if loop_rep:
                loop_cm.__exit__(None, None, None)

            # ---- bag sums: 4 chunks per bag ----
            lgt = constp.tile([C, ROWS // 2048], f32)
            nc.vector.reduce_sum(lgt[:], psb[:, :].rearrange("p (b f) -> p b f", f=4),
                                 axis=mybir.AxisListType.X)
            nc.sync.dma_start(lg[:, :], lgt[:])

    nc.compile()
    _built[loop_rep] = nc
    return nc


def _host_prep(features, attention, W1, b1, W2, b2):
    features = np.ascontiguousarray(np.asarray(features, dtype=np.float32))
    att = np.ascontiguousarray(np.asarray(attention, dtype=np.float32).reshape(-1))
    w1b = np.asarray(W1, dtype=np.float32).astype(ml_dtypes.bfloat16)
    w2b = np.asarray(W2, dtype=np.float32).astype(ml_dtypes.bfloat16)
    b1f = np.asarray(b1, dtype=np.float32)
    b2f = np.asarray(b2, dtype=np.float32)
    in_maps = []
    for cix in range(N_CORES):
        s = slice(cix * ROWS, (cix + 1) * ROWS)
        in_maps.append({"x": features[s], "att": att[s], "w1": w1b, "b1": b1f,
                        "w2": w2b, "b2": b2f,
                        "chain": np.zeros((1, 1), np.float32)})
    return in_maps


def kernel(features, attention, bag_sizes, W1, b1, W2, b2):
    nc = build()
    in_maps = _host_prep(features, attention, W1, b1, W2, b2)
    results = run_bass_kernel_spmd(nc, in_maps, core_ids=list(range(N_CORES))).results
    patch_logits = np.concatenate([r["pl"].T for r in results], axis=0)
    bag_sizes = np.asarray(bag_sizes).reshape(-1).astype(np.int64)
    if bag_sizes.size and (bag_sizes == 2048).all():
        logits = np.concatenate([r["lg"].T for r in results], axis=0)
    else:
        # general (never hit for this problem's spec): host segment-sum
        offsets = np.zeros(bag_sizes.size, dtype=np.int64)
        np.cumsum(bag_sizes[:-1], out=offsets[1:])
        logits = np.add.reduceat(patch_logits.astype(np.float64), offsets, axis=0).astype(np.float32)
    return logits.astype(np.float32), patch_logits.astype(np.float32)


# revision 32
# speedup vs baseline: 317.6668x; 1.0675x over previous
"""AdditiveClassifier (attention-gated MLP + per-bag segment sum) on 8 TRN2 cores.

Sharding: patch dim N=131072 split as 16384 rows/core (whole bags stay on one
core), MLP weights replicated, outputs gathered on host by concatenation (no
collectives). Per-core dataflow, in chunks of 512 rows:
  1. SWDGE DMA loads the chunk in natural layout, casting fp32->bf16 in-flight
     (round-to-nearest).
  2. PE transpose-mode matmuls produce xT 128x128 blocks in PSUM (4 blocks per
     PSUM tile); one wide DVE/ACT copy per kt moves them to SBUF. The gating
     scale commutes past the matmul (it is applied per-row later), so x is
     multiplied by attention only after L1.
  3. L1: W1-stationary bf16 matmuls accumulate h^T [128h, 512rows] in PSUM
     (8 k-tiles x 4 m-tiles, N=512 moving).
  4. DVE multiplies by attention (broadcast across partitions), ACT applies
     per-partition bias + ReLU and casts to bf16.
  5. L2: 4 accumulating [128,2]x[128,512] matmuls -> patch_logits^T [2, 512];
     DVE adds b2.
  6. patch_logits is stored transposed [2, rows] (contiguous per-partition
     DMA); the host transposes back during the gather. Bag sums are free-dim
     reduce_sums (4 chunk-partials per 2048-row bag).
"""
import os
import numpy as np
import ml_dtypes

import concourse.bacc as bacc
import concourse.mybir as mybir
import concourse.tile as tile
from concourse.bass_utils import run_bass_kernel_spmd
from concourse.masks import make_identity

N_CORES = 8
N, D, H, C = 131072, 1024, 512, 2
ROWS = N // N_CORES          # 16384 rows per core
R = 512                      # rows per chunk (PSUM free-dim limit for fp32)
CHUNKS = ROWS // R           # 32
KO1 = D // 128               # 8  k-tiles for L1
MO1 = H // 128               # 4  m-tiles for L1 (= k-tiles for L2)
RT = R // 128                # 4  row-tiles per chunk

CAST_MODE = os.environ.get("AC_CAST_MODE", "swdge")   # swdge | dve
N_XBAR_KT = int(os.environ.get("AC_XBAR_KT", "0"))    # kt groups on the X-bar path

_built = {}


def build(loop_rep=0):
    """loop_rep>0: wrap the chunk loop in a constant-bound For_i repeating it
    loop_rep times (on-device timing); 0: plain production build."""
    if loop_rep in _built:
        return _built[loop_rep]
    nc = bacc.Bacc("TRN2", target_bir_lowering=False, debug=False,
                   num_devices=N_CORES)
    f32, bf16 = mybir.dt.float32, mybir.dt.bfloat16

    x = nc.declare_dram_parameter("x", [ROWS, D], f32, isOutput=False)
    att = nc.declare_dram_parameter("att", [ROWS], f32, isOutput=False)
    w1 = nc.declare_dram_parameter("w1", [D, H], bf16, isOutput=False)
    b1 = nc.declare_dram_parameter("b1", [H], f32, isOutput=False)
    w2 = nc.declare_dram_parameter("w2", [H, C], bf16, isOutput=False)
    b2 = nc.declare_dram_parameter("b2", [C], f32, isOutput=False)
    # outputs stored transposed ([C, rows]) for contiguous per-partition DMA;
    # host transposes back during the gather
    pl = nc.declare_dram_parameter("pl", [C, ROWS], f32, isOutput=True)
    lg = nc.declare_dram_parameter("lg", [C, ROWS // 2048], f32, isOutput=True)
    chain_in = nc.declare_dram_parameter("chain", [1, 1], f32, isOutput=False)
    chain_out = nc.declare_dram_parameter("chain_o", [1, 1], f32, isOutput=True)

    x_chunks = x[:, :].rearrange("(c rt p) d -> c p rt d", p=128, rt=RT)

    with tile.TileContext(nc) as tc:
        with (
            tc.tile_pool(name="const", bufs=1) as constp,
            tc.tile_pool(name="nat", bufs=3) as natp,
            tc.tile_pool(name="natf", bufs=2) as natfp,
            tc.tile_pool(name="xt", bufs=3) as xtp,
            tc.tile_pool(name="ab", bufs=2) as abp,
            tc.tile_pool(name="tg", bufs=2) as tgp,
            tc.tile_pool(name="ht", bufs=8) as htp,
            tc.tile_pool(name="plt", bufs=2) as pltp,
            tc.tile_pool(name="ps1", bufs=4, space="PSUM") as ps1p,
            tc.tile_pool(name="ps2", bufs=1, space="PSUM") as ps2p,
            tc.tile_pool(name="tps", bufs=3, space="PSUM") as tpsp,
        ):
            # ---- preload constants ----
            w1sb = constp.tile([128, KO1, H], bf16)
            for ko in range(KO1):
                nc.scalar.dma_start(w1sb[:, ko, :], w1[ko * 128:(ko + 1) * 128, :])
            b1sb = constp.tile([128, MO1], f32)
            nc.scalar.dma_start(b1sb[:], b1[:].rearrange("(mo p) -> p mo", p=128))
            w2sb = constp.tile([128, MO1, C], bf16)
            nc.scalar.dma_start(w2sb[:], w2[:, :].rearrange("(ko ki) c -> ki ko c", ki=128))
            b2sb = constp.tile([C, 1], f32)
            nc.scalar.dma_start(b2sb[:], b2[:].rearrange("(c one) -> c one", one=1))
            ident = constp.tile([128, 128], bf16)
            make_identity(nc, ident[:])
            psb = constp.tile([C, CHUNKS], f32)
            cht = constp.tile([1, 1], f32)
            nc.scalar.dma_start(cht[:], chain_in[:, :])
            nc.scalar.dma_start(chain_out[:, :], cht[:])

            if loop_rep:
                loop_cm = tc.For_i(0, loop_rep, 1)
                loop_cm.__enter__()

            for c in range(CHUNKS):
                r0 = c * R
                # ---- natural-layout load with fp32->bf16 cast ----
                xnat = natp.tile([128, RT, D], bf16, tag="nat")
                if CAST_MODE == "swdge":
                    nc.gpsimd.dma_start(xnat[:], x_chunks[c])
                else:
                    xf32 = natfp.tile([128, RT, D], f32, tag="natf")
                    nc.sync.dma_start(xf32[:], x_chunks[c])
                    nc.vector.tensor_copy(xnat[:], xf32[:])
                # ---- on-chip transpose to xT [128d, KO1, R] via PE ----
                # 4 row-tile transposes of one kt land in one PSUM tile, then a
                # single wide copy moves them to SBUF (alternating DVE/ACT).
                xt = xtp.tile([128, KO1, R], bf16, tag="xt")
                for kt in range(KO1):
                    if kt >= KO1 - N_XBAR_KT:
                        # X-bar path: straight SBUF->SBUF transposed DMA
                        for rt in range(RT):
                            nc.scalar.dma_start(
                                xt[:, kt, rt * 128:(rt + 1) * 128],
                                xnat[:, rt, kt * 128:(kt + 1) * 128],
                                transpose=True)
                        continue
                    tps = tpsp.tile([128, RT, 128], bf16, tag="tps")
                    for rt in range(RT):
                        nc.tensor.matmul(
                            tps[:, rt, :], xnat[:, rt, kt * 128:(kt + 1) * 128],
                            ident[:], is_transpose=True, skip_group_check=True)
                    dst = xt[:, kt, :]
                    if kt % 2:
                        nc.scalar.activation(
                            dst, tps[:], mybir.ActivationFunctionType.Copy)
                    else:
                        nc.vector.tensor_copy(dst, tps[:])
                # ---- attention broadcast [128, R] ----
                ab = abp.tile([128, R], f32, tag="ab")
                nc.sync.dma_start(ab[:], att[r0:r0 + R].partition_broadcast(128))
                # ---- L1 + gate + ReLU -> hT bf16 tiles ----
                ht = []
                for mo in range(MO1):
                    ps = ps1p.tile([128, R], f32, tag="ps")
                    for ko in range(KO1):
                        nc.tensor.matmul(
                            ps[:],
                            w1sb[:, ko, mo * 128:(mo + 1) * 128],
                            xt[:, ko, :],
                            start=(ko == 0), stop=(ko == KO1 - 1),
                        )
                    tg = tgp.tile([128, R], f32, tag="tg")
                    nc.vector.tensor_tensor(tg[:], ps[:], ab[:], op=mybir.AluOpType.mult)
                    h = htp.tile([128, R], bf16, tag="ht")
                    nc.scalar.activation(h[:], tg[:], mybir.ActivationFunctionType.Relu,
                                         bias=b1sb[:, mo:mo + 1])
                    ht.append(h)
                # ---- L2 -> patch_logits^T [2, R] ----
                ps2 = ps2p.tile([C, R], f32, tag="ps2")
                for ko in range(MO1):
                    nc.tensor.matmul(ps2[:], w2sb[:, ko, :], ht[ko][:],
                                     start=(ko == 0), stop=(ko == MO1 - 1))
                plt = pltp.tile([C, R], f32, tag="plt")
                nc.vector.tensor_scalar_add(plt[:], ps2[:], b2sb[:])
                nc.sync.dma_start(pl[:, r0:r0 + R], plt[:])
                # per-chunk bag partial
                nc.vector.reduce_sum(psb[:, c:c + 1], plt[:],
                                     axis=mybir.AxisListType.X)

            if loop_rep:
                loop_cm.__exit__(None, None, None)

            # ---- bag sums: 4 chunks per bag ----
            lgt = constp.tile([C, ROWS // 2048], f32)
            nc.vector.reduce_sum(lgt[:], psb[:, :].rearrange("p (b f) -> p b f", f=4),
                                 axis=mybir.AxisListType.X)
            nc.sync.dma_start(lg[:, :], lgt[:])

    nc.compile()
    _built[loop_rep] = nc
    return nc


def _host_prep(features, attention, W1, b1, W2, b2):
    features = np.ascontiguousarray(np.asarray(features, dtype=np.float32))
    att = np.ascontiguousarray(np.asarray(attention, dtype=np.float32).reshape(-1))
    w1b = np.asarray(W1, dtype=np.float32).astype(ml_dtypes.bfloat16)
    w2b = np.asarray(W2, dtype=np.float32).astype(ml_dtypes.bfloat16)
    b1f = np.asarray(b1, dtype=np.float32)
    b2f = np.asarray(b2, dtype=np.float32)
    in_maps = []
    for cix in range(N_CORES):
        s = slice(cix * ROWS, (cix + 1) * ROWS)
        in_maps.append({"x": features[s], "att": att[s], "w1": w1b, "b1": b1f,
                        "w2": w2b, "b2": b2f,
                        "chain": np.zeros((1, 1), np.float32)})
    return in_maps


def kernel(features, attention, bag_sizes, W1, b1, W2, b2):
    nc = build()
    in_maps = _host_prep(features, attention, W1, b1, W2, b2)
    results = run_bass_kernel_spmd(nc, in_maps, core_ids=list(range(N_CORES))).results
    patch_logits = np.concatenate([r["pl"].T for r in results], axis=0)
    bag_sizes = np.asarray(bag_sizes).reshape(-1).astype(np.int64)
    if bag_sizes.size and (bag_sizes == 2048).all():
        logits = np.concatenate([r["lg"].T for r in results], axis=0)
    else:
        # general (never hit for this problem's spec): host segment-sum
        offsets = np.zeros(bag_sizes.size, dtype=np.int64)
        np.cumsum(bag_sizes[:-1], out=offsets[1:])
        logits = np.add.reduceat(patch_logits.astype(np.float64), offsets, axis=0).astype(np.float32)
    return logits.astype(np.float32), patch_logits.astype(np.float32)
